# revision 15
# baseline (speedup 1.0000x reference)
"""Trainium2 Bass kernel for nn_EntityEncoder (multi-hot embedding bag + MLP head).

Strategy: vocab (E) sharding across 8 cores.

Host prep per core:
  - x slice [512, 6250] -> transposed/padded [128, 50*512] fp8_e4m3 (values
    0/1 exact; 3.28 MB instead of 13.1 MB int32),
  - emb shard + ones column [128, 50*129] bf16 (1.66 MB) -- the ones column
    makes the per-path COUNT fall out of the same matmuls as column 128,
  - params with LN gamma/beta folded into the linear weights.

Device per core:
  - main GEMM oriented out[bp, 129]: for each of 50 K=128 subtiles and each
    of 4 bp-quarters, matmul(psum_q, lhsT=x_tile, rhs=[emb|1]) accumulates
    sums AND counts in 4 PSUM banks. fp8 lhsT x bf16 rhs, fp32 accum.
  - one ReduceScatter(add) over [512, 129] f32 partials: each core receives
    the fully-summed [64, 129] for its own 64 paths (its 4 batches).
  - head in [b,h]<->[h,b] alternating layout: divide-by-count and path-mean
    fused into one tiny matmul via a rec-scaled block-mask; LN stats via
    bn_stats/bn_aggr; bias+relu and bn as per-partition tensor_scalar ops.
  - a tiny warmup ReduceScatter issued at priority 0 absorbs the ncfw
    init/barrier cost under the main loop.
"""

import numpy as np

B, P, E, H = 32, 16, 50000, 128
NCORES = 8
BP = B * P                  # 512
E_SH = E // NCORES          # 6250
SUB = 128
E_PAD = 6400
NSUB = E_PAD // SUB         # 50
NCH = 5                     # x DMA chunks
SPC = NSUB // NCH           # 10 subtiles per chunk
EPS = 1e-5
NB = BP // NCORES           # 64 local paths
BL = B // NCORES            # 4 local batches

# packed params [128, NPAR] f32:
# col 0 b1f, 1 b2f, 2 bn1_g', 3 bn1_b, 4 bn2_g', 5 bn2_b,
# col 6 eps1 (EPS*P*P), 7 eps2 (EPS), cols 8:12 M block-mask (rows 0:64),
# cols 12:140 (w1*ln1_g)^T, cols 140:268 (w2*ln2_g)^T
NPAR = 268

_cached = {}


def _build(lean1=False, lean2=False):
    import concourse.bacc as bacc
    import concourse.mybir as mybir
    import concourse.tile as tile
    from concourse import masks

    f32 = mybir.dt.float32
    bf16 = mybir.dt.bfloat16
    fp8 = mybir.dt.float8e4

    nc = bacc.Bacc("TRN2", target_bir_lowering=False, debug=False,
                   num_devices=NCORES)

    x_d = nc.dram_tensor("x", [SUB, NSUB * BP], fp8, kind="ExternalInput")
    emb_d = nc.dram_tensor("emb", [SUB, NSUB * 129], bf16,
                           kind="ExternalInput")
    par_d = nc.dram_tensor("par", [128, NPAR], f32, kind="ExternalInput")
    out_d = nc.dram_tensor("out", [BL, H], f32, kind="ExternalOutput")

    groups = [list(range(NCORES))]

    with tile.TileContext(nc) as tc:
        with tc.tile_pool(name="const", bufs=1) as constp, \
             tc.tile_pool(name="xin", bufs=3) as xin, \
             tc.tile_pool(name="head", bufs=1) as head, \
             tc.tile_pool(name="ps_acc", bufs=1, space="PSUM") as ps_acc, \
             tc.tile_pool(name="ps_head", bufs=4, space="PSUM") as ps_head, \
             tc.tile_pool(name="dram", bufs=1) as dram:

            # ---- warmup collective: pay ncfw init early, overlapped.
            # The whole chain (memset -> SWDGE stage -> doorbell) lives on
            # the gpsimd queue at priority 0, so the doorbell fires as soon
            # as the preamble drains -- no cross-engine dependencies.
            with tc.high_priority():
                wz = constp.tile([8, 8], f32)
                nc.gpsimd.memset(wz[:], 0.0)
                ccw_in = dram.tile([8, 8], f32, space="DRAM")
                ccw_out = dram.tile([1, 8], f32, space="DRAM")
                nc.gpsimd.dma_start(ccw_in[:], wz[:])
                nc.gpsimd.collective_compute(
                    "ReduceScatter",
                    mybir.AluOpType.add,
                    replica_groups=groups,
                    ins=[ccw_in[:].opt()],
                    outs=[ccw_out[:].opt()],
                )

            # ---- constants ----
            ident = constp.tile([128, 128], f32)
            masks.make_identity(nc, ident[:])
            par = constp.tile([128, NPAR], f32)
            nc.scalar.dma_start(par[:], par_d[:, :])

            # resident [emb | ones] bf16, 10 chunks (5 subtiles each) so the
            # first matmuls are gated on ~0.17 MB, not half the table
            emb_b = constp.tile([SUB, NSUB * 129], bf16)
            EC = NSUB * 129 // 10
            for k in range(10):
                nc.scalar.dma_start(emb_b[:, k * EC:(k + 1) * EC],
                                    emb_d[:, k * EC:(k + 1) * EC])

            # preload Sqrt ACT table off the critical path
            warm = constp.tile([1, 1], f32)
            nc.scalar.activation(warm[:], par[0:1, 7:8],
                                 mybir.ActivationFunctionType.Sqrt,
                                 bias=par[0:1, 7:8], scale=1.0)

            # ---- main GEMM: out[bp, 129] in 4 PSUM banks ----
            ps = [ps_acc.tile([128, 512], f32, name=f"acc{q}")
                  for q in range(4)]
            for t in range(NCH):
                xt = xin.tile([SUB, SPC * BP], fp8, tag="xt", name=f"xt{t}")
                nc.sync.dma_start(
                    xt[:], x_d[:, t * SPC * BP:(t + 1) * SPC * BP])
                for j in range(SPC):
                    g = t * SPC + j
                    rhs = emb_b[:, g * 129:(g + 1) * 129]
                    for q in range(4):
                        nc.tensor.matmul(
                            ps[q][:, 0:129],
                            xt[:, j * BP + q * 128: j * BP + (q + 1) * 128],
                            rhs,
                            start=(g == 0), stop=(g == NSUB - 1))

            # ---- stage partials + ReduceScatter ----
            stage = head.tile([128, 4 * 129], f32)
            for q in range(4):
                if q % 2 == 0:
                    nc.vector.tensor_copy(stage[:, q * 129:(q + 1) * 129],
                                          ps[q][:, 0:129])
                else:
                    nc.scalar.copy(stage[:, q * 129:(q + 1) * 129],
                                   ps[q][:, 0:129])
            cc_in = dram.tile([BP, 129], f32, space="DRAM")
            cc_out = dram.tile([NB, 129], f32, space="DRAM")
            nc.sync.dma_start(
                cc_in[:].rearrange("(q p) c -> p q c", p=128),
                stage[:].rearrange("p (q c) -> p q c", c=129))
            nc.gpsimd.collective_compute(
                "ReduceScatter",
                mybir.AluOpType.add,
                replica_groups=groups,
                ins=[cc_in[:].opt()],
                outs=[cc_out[:].opt()],
            )

            # ---- head on [64, 129] totals ----
            S = head.tile([NB, 129], f32)
            nc.sync.dma_start(S[:], cc_out[:])

            rec = head.tile([NB, 1], f32)
            nc.vector.reciprocal(rec[:], S[:, 128:129])
            R = head.tile([NB, BL], f32)
            nc.vector.tensor_scalar(
                out=R[:], in0=par[0:NB, 8:12], scalar1=rec[:, 0:1],
                scalar2=None, op0=mybir.AluOpType.mult)

            # x0[b, h] = sum_p sums[p, h] / cnt[p]   (= P * mean; LN-invariant)
            x0_ps = ps_head.tile([BL, 128], f32, tag="psh", name="x0")
            nc.tensor.matmul(x0_ps[:], R[:], S[:, 0:128],
                             start=True, stop=True)

            def layer_norm(x_ps, eps_col, name):
                # x_ps: [BL, 128] PSUM -> xn [BL, 128] SBUF
                st6 = head.tile([BL, 6], f32, tag=f"{name}_st6")
                nc.vector.bn_stats(st6[:], x_ps[:])
                mv = head.tile([BL, 2], f32, tag=f"{name}_mv")
                nc.vector.bn_aggr(mv[:], st6[:])
                sd = head.tile([BL, 1], f32, tag=f"{name}_sd")
                nc.scalar.activation(sd[:], mv[:, 1:2],
                                     mybir.ActivationFunctionType.Sqrt,
                                     bias=par[0:BL, eps_col:eps_col + 1],
                                     scale=1.0)
                rstd = head.tile([BL, 1], f32, tag=f"{name}_rstd")
                nc.vector.reciprocal(rstd[:], sd[:])
                xn = head.tile([BL, 128], f32, tag=f"{name}_xn")
                nc.vector.tensor_scalar(
                    out=xn[:], in0=x_ps[:],
                    scalar1=mv[:, 0:1], scalar2=rstd[:, 0:1],
                    op0=mybir.AluOpType.subtract, op1=mybir.AluOpType.mult)
                return xn

            def linear_relu_bn(xn, w_lo, b_col, bng_col, bnb_col, lean,
                               name):
                # xn [BL, 128] -> z [128, BL]. When `lean`, the eval-BN
                # scale is folded into the weights host-side and the bn
                # op is dropped.
                xt_ps = ps_head.tile([128, BL], f32, tag="psh",
                                     name=f"{name}_xt")
                nc.tensor.transpose(xt_ps[:], xn[:], ident[0:BL, 0:BL])
                xt_sb = head.tile([128, BL], f32, tag=f"{name}_xts")
                nc.vector.tensor_copy(xt_sb[:], xt_ps[:])
                y_ps = ps_head.tile([128, BL], f32, tag="psh",
                                    name=f"{name}_y")
                nc.tensor.matmul(y_ps[:], par[:, w_lo:w_lo + 128], xt_sb[:],
                                 start=True, stop=True)
                y = head.tile([128, BL], f32, tag=f"{name}_relu")
                nc.vector.tensor_scalar(
                    out=y[:], in0=y_ps[:],
                    scalar1=par[:, b_col:b_col + 1], scalar2=0.0,
                    op0=mybir.AluOpType.add, op1=mybir.AluOpType.max)
                if lean:
                    return y
                z = head.tile([128, BL], f32, tag=f"{name}_bn")
                nc.vector.tensor_scalar(
                    out=z[:], in0=y[:],
                    scalar1=par[:, bng_col:bng_col + 1],
                    scalar2=par[:, bnb_col:bnb_col + 1],
                    op0=mybir.AluOpType.mult, op1=mybir.AluOpType.add)
                return z

            h1 = layer_norm(x0_ps, 6, "ln1")
            z1 = linear_relu_bn(h1, 12, 0, 2, 3, lean1, "l1")
            z1t_ps = ps_head.tile([BL, 128], f32, tag="psh", name="z1t")
            nc.tensor.transpose(z1t_ps[:], z1[:], ident[:, :])
            h2 = layer_norm(z1t_ps, 7, "ln2")
            z2 = linear_relu_bn(h2, 140, 1, 4, 5, lean2, "l2")

            out_ps = ps_head.tile([BL, 128], f32, tag="psh", name="outT")
            nc.tensor.transpose(out_ps[:], z2[:], ident[:, :])
            out_sb = head.tile([BL, 128], f32)
            nc.vector.tensor_copy(out_sb[:], out_ps[:])
            nc.scalar.dma_start(out_d[:, :], out_sb[:])

    nc.compile()
    return nc


def _prepare_in_maps(inputs):
    import ml_dtypes

    x = np.asarray(inputs["inputs"])
    emb = np.asarray(inputs["emb"], dtype=np.float32)
    w1 = np.asarray(inputs["w1"], dtype=np.float32)
    b1 = np.asarray(inputs["b1"], dtype=np.float32)
    w2 = np.asarray(inputs["w2"], dtype=np.float32)
    b2 = np.asarray(inputs["b2"], dtype=np.float32)
    ln1_g = np.asarray(inputs["ln1_g"], np.float32)
    ln1_b = np.asarray(inputs["ln1_b"], np.float32)
    ln2_g = np.asarray(inputs["ln2_g"], np.float32)
    ln2_b = np.asarray(inputs["ln2_b"], np.float32)

    par = np.zeros((128, NPAR), dtype=np.float32)
    # y = W @ (g*xn + b) + b1 = (W*g) @ xn + (W@b + b1)
    w1f = w1 * ln1_g[None, :]
    b1f = b1 + w1 @ ln1_b
    w2f = w2 * ln2_g[None, :]
    b2f = b2 + w2 @ ln2_b
    bn1_g = np.asarray(inputs["bn1_g"], np.float32) / np.sqrt(
        np.float32(1.0) + np.float32(EPS))
    bn1_b = np.asarray(inputs["bn1_b"], np.float32)
    bn2_g = np.asarray(inputs["bn2_g"], np.float32) / np.sqrt(
        np.float32(1.0) + np.float32(EPS))
    bn2_b = np.asarray(inputs["bn2_b"], np.float32)
    # g>0 and b==0 lets BN fold into the preceding linear: g*relu(Wx+b) =
    # relu((g*W)x + g*b)
    lean1 = bool((bn1_g > 0).all() and (bn1_b == 0).all())
    lean2 = bool((bn2_g > 0).all() and (bn2_b == 0).all())
    if lean1:
        w1f = w1f * bn1_g[:, None]
        b1f = b1f * bn1_g
    if lean2:
        w2f = w2f * bn2_g[:, None]
        b2f = b2f * bn2_g
    par[:, 0] = b1f
    par[:, 1] = b2f
    par[:, 2] = bn1_g
    par[:, 3] = bn1_b
    par[:, 4] = bn2_g
    par[:, 5] = bn2_b
    par[:, 6] = EPS * P * P
    par[:, 7] = EPS
    for i in range(NB):
        par[i, 8 + i // P] = 1.0
    par[:, 12:140] = w1f.T
    par[:, 140:268] = w2f.T

    x_flat = np.asarray(x).reshape(BP, E)
    in_maps = []
    for c in range(NCORES):
        lo = c * E_SH
        seg_t = np.zeros((E_PAD, BP), dtype=np.int8)
        seg_t[:E_SH] = (x_flat[:, lo:lo + E_SH].T == 1)
        x_sh = np.ascontiguousarray(
            seg_t.reshape(NSUB, SUB, BP).transpose(1, 0, 2)
        ).reshape(SUB, NSUB * BP).astype(ml_dtypes.float8_e4m3)
        seg_e = np.zeros((E_PAD, 129), dtype=np.float32)
        seg_e[:E_SH, 0:128] = emb[lo:lo + E_SH, :]
        if c == 0:
            seg_e[0, 0:128] = 0.0   # padding_idx=0
        seg_e[:, 128] = 1.0         # count column
        emb_sh = np.ascontiguousarray(
            seg_e.reshape(NSUB, SUB, 129).transpose(1, 0, 2)
        ).reshape(SUB, NSUB * 129).astype(ml_dtypes.bfloat16)
        in_maps.append({"x": x_sh, "emb": emb_sh, "par": par})
    return in_maps, lean1, lean2


def _run(inputs, trace=False):
    from concourse.bass_utils import run_bass_kernel_spmd

    in_maps, lean1, lean2 = _prepare_in_maps(inputs)
    key = ("nc", lean1, lean2)
    if key not in _cached:
        _cached[key] = _build(lean1, lean2)
    nc = _cached[key]
    res = run_bass_kernel_spmd(
        nc, in_maps, core_ids=list(range(NCORES)), trace=trace)
    out = np.concatenate(
        [np.asarray(res.results[c]["out"]) for c in range(NCORES)], axis=0)
    return out, res.exec_time_ns


def kernel(**inputs) -> np.ndarray:
    out, _ = _run(inputs, trace=False)
    return out


# revision 16
# speedup vs baseline: 1.2142x; 1.2142x over previous
"""Trainium2 Bass kernel for nn_EntityEncoder (multi-hot embedding bag + MLP head).

Strategy: vocab (E) sharding across 8 cores.

Host prep per core:
  - x slice [512, 6250] -> transposed/padded [128, 50*512] fp8_e4m3 (values
    0/1 exact; 3.28 MB instead of 13.1 MB int32),
  - emb shard + ones column [128, 50*129] bf16 (1.66 MB) -- the ones column
    makes the per-path COUNT fall out of the same matmuls as column 128,
  - params with LN gamma/beta folded into the linear weights.

Device per core:
  - main GEMM oriented out[bp, 129]: for each of 50 K=128 subtiles and each
    of 4 bp-quarters, matmul(psum_q, lhsT=x_tile, rhs=[emb|1]) accumulates
    sums AND counts in 4 PSUM banks. fp8 lhsT x bf16 rhs, fp32 accum.
  - one ReduceScatter(add) over [512, 129] f32 partials: each core receives
    the fully-summed [64, 129] for its own 64 paths (its 4 batches).
  - head in [b,h]<->[h,b] alternating layout: divide-by-count and path-mean
    fused into one tiny matmul via a rec-scaled block-mask; LN stats via
    bn_stats/bn_aggr; bias+relu and bn as per-partition tensor_scalar ops.
  - a tiny warmup ReduceScatter issued at priority 0 absorbs the ncfw
    init/barrier cost under the main loop.
"""

import numpy as np

B, P, E, H = 32, 16, 50000, 128
NCORES = 8
BP = B * P                  # 512
E_SH = E // NCORES          # 6250
SUB = 128
E_PAD = 6400
NSUB = E_PAD // SUB         # 50
NCH = 5                     # x DMA chunks
SPC = NSUB // NCH           # 10 subtiles per chunk
EPS = 1e-5
NB = BP // NCORES           # 64 local paths
BL = B // NCORES            # 4 local batches

# packed params [128, NPAR] f32:
# col 0 b1f, 1 b2f, 2 bn1_g', 3 bn1_b, 4 bn2_g', 5 bn2_b,
# col 6 eps1 (EPS*P*P), 7 eps2 (EPS), cols 8:12 M block-mask (rows 0:64),
# cols 12:140 (w1*ln1_g)^T, cols 140:268 (w2*ln2_g)^T
NPAR = 268

_cached = {}


def _build(lean1=False, lean2=False):
    import concourse.bacc as bacc
    import concourse.mybir as mybir
    import concourse.tile as tile
    from concourse import masks

    f32 = mybir.dt.float32
    bf16 = mybir.dt.bfloat16
    fp8 = mybir.dt.float8e4

    nc = bacc.Bacc("TRN2", target_bir_lowering=False, debug=False,
                   num_devices=NCORES)

    x_d = nc.dram_tensor("x", [SUB, NSUB * BP], fp8, kind="ExternalInput")
    emb_d = nc.dram_tensor("emb", [SUB, NSUB * 129], bf16,
                           kind="ExternalInput")
    par_d = nc.dram_tensor("par", [128, NPAR], f32, kind="ExternalInput")
    out_d = nc.dram_tensor("out", [BL, H], f32, kind="ExternalOutput")

    groups = [list(range(NCORES))]

    with tile.TileContext(nc) as tc:
        with tc.tile_pool(name="const", bufs=1) as constp, \
             tc.tile_pool(name="xin", bufs=3) as xin, \
             tc.tile_pool(name="head", bufs=1) as head, \
             tc.tile_pool(name="ps_acc", bufs=1, space="PSUM") as ps_acc, \
             tc.tile_pool(name="ps_head", bufs=4, space="PSUM") as ps_head, \
             tc.tile_pool(name="dram", bufs=1) as dram:

            # ---- early Tensor op: the ncfw auto-barrier for the (single)
            # collective is triggered from the Tensor queue; a tiny
            # zero-dependency matmul at priority 0 lets that trigger fire
            # right after the preamble instead of waiting for the first
            # DMA-gated real matmul (~7us earlier barrier start).
            with tc.high_priority():
                wz = constp.tile([1, 1], f32)
                nc.vector.memset(wz[:], 0.0)
                dmy_ps = ps_head.tile([1, 1], f32, tag="psh", name="dmy")
                nc.tensor.matmul(dmy_ps[:], wz[:], wz[:],
                                 start=True, stop=True)

            # ---- constants ----
            ident = constp.tile([128, 128], f32)
            masks.make_identity(nc, ident[:])
            par = constp.tile([128, NPAR], f32)
            nc.scalar.dma_start(par[:], par_d[:, :])

            # resident [emb | ones] bf16, 10 chunks (5 subtiles each) so the
            # first matmuls are gated on ~0.17 MB, not half the table
            emb_b = constp.tile([SUB, NSUB * 129], bf16)
            EC = NSUB * 129 // 10
            for k in range(10):
                nc.scalar.dma_start(emb_b[:, k * EC:(k + 1) * EC],
                                    emb_d[:, k * EC:(k + 1) * EC])

            # preload Sqrt ACT table off the critical path
            warm = constp.tile([1, 1], f32)
            nc.scalar.activation(warm[:], par[0:1, 7:8],
                                 mybir.ActivationFunctionType.Sqrt,
                                 bias=par[0:1, 7:8], scale=1.0)

            # ---- main GEMM: out[bp, 129] in 4 PSUM banks ----
            ps = [ps_acc.tile([128, 512], f32, name=f"acc{q}")
                  for q in range(4)]
            for t in range(NCH):
                xt = xin.tile([SUB, SPC * BP], fp8, tag="xt", name=f"xt{t}")
                nc.sync.dma_start(
                    xt[:], x_d[:, t * SPC * BP:(t + 1) * SPC * BP])
                for j in range(SPC):
                    g = t * SPC + j
                    rhs = emb_b[:, g * 129:(g + 1) * 129]
                    for q in range(4):
                        nc.tensor.matmul(
                            ps[q][:, 0:129],
                            xt[:, j * BP + q * 128: j * BP + (q + 1) * 128],
                            rhs,
                            start=(g == 0), stop=(g == NSUB - 1))

            # ---- stage partials + ReduceScatter ----
            stage = head.tile([128, 4 * 129], f32)
            for q in range(4):
                if q % 2 == 0:
                    nc.vector.tensor_copy(stage[:, q * 129:(q + 1) * 129],
                                          ps[q][:, 0:129])
                else:
                    nc.scalar.copy(stage[:, q * 129:(q + 1) * 129],
                                   ps[q][:, 0:129])
            cc_in = dram.tile([BP, 129], f32, space="DRAM")
            cc_out = dram.tile([NB, 129], f32, space="DRAM")
            nc.sync.dma_start(
                cc_in[:].rearrange("(q p) c -> p q c", p=128),
                stage[:].rearrange("p (q c) -> p q c", c=129))
            nc.gpsimd.collective_compute(
                "ReduceScatter",
                mybir.AluOpType.add,
                replica_groups=groups,
                ins=[cc_in[:].opt()],
                outs=[cc_out[:].opt()],
            )

            # ---- head on [64, 129] totals ----
            S = head.tile([NB, 129], f32)
            nc.sync.dma_start(S[:], cc_out[:])

            rec = head.tile([NB, 1], f32)
            nc.vector.reciprocal(rec[:], S[:, 128:129])
            R = head.tile([NB, BL], f32)
            nc.vector.tensor_scalar(
                out=R[:], in0=par[0:NB, 8:12], scalar1=rec[:, 0:1],
                scalar2=None, op0=mybir.AluOpType.mult)

            # x0[b, h] = sum_p sums[p, h] / cnt[p]   (= P * mean; LN-invariant)
            x0_ps = ps_head.tile([BL, 128], f32, tag="psh", name="x0")
            nc.tensor.matmul(x0_ps[:], R[:], S[:, 0:128],
                             start=True, stop=True)

            def layer_norm(x_ps, eps_col, name):
                # x_ps: [BL, 128] PSUM -> xn [BL, 128] SBUF
                st6 = head.tile([BL, 6], f32, tag=f"{name}_st6")
                nc.vector.bn_stats(st6[:], x_ps[:])
                mv = head.tile([BL, 2], f32, tag=f"{name}_mv")
                nc.vector.bn_aggr(mv[:], st6[:])
                sd = head.tile([BL, 1], f32, tag=f"{name}_sd")
                nc.scalar.activation(sd[:], mv[:, 1:2],
                                     mybir.ActivationFunctionType.Sqrt,
                                     bias=par[0:BL, eps_col:eps_col + 1],
                                     scale=1.0)
                rstd = head.tile([BL, 1], f32, tag=f"{name}_rstd")
                nc.vector.reciprocal(rstd[:], sd[:])
                xn = head.tile([BL, 128], f32, tag=f"{name}_xn")
                nc.vector.tensor_scalar(
                    out=xn[:], in0=x_ps[:],
                    scalar1=mv[:, 0:1], scalar2=rstd[:, 0:1],
                    op0=mybir.AluOpType.subtract, op1=mybir.AluOpType.mult)
                return xn

            def linear_relu_bn(xn, w_lo, b_col, bng_col, bnb_col, lean,
                               name):
                # xn [BL, 128] -> z [128, BL]. When `lean`, the eval-BN
                # scale is folded into the weights host-side and the bn
                # op is dropped.
                xt_ps = ps_head.tile([128, BL], f32, tag="psh",
                                     name=f"{name}_xt")
                nc.tensor.transpose(xt_ps[:], xn[:], ident[0:BL, 0:BL])
                xt_sb = head.tile([128, BL], f32, tag=f"{name}_xts")
                nc.vector.tensor_copy(xt_sb[:], xt_ps[:])
                y_ps = ps_head.tile([128, BL], f32, tag="psh",
                                    name=f"{name}_y")
                nc.tensor.matmul(y_ps[:], par[:, w_lo:w_lo + 128], xt_sb[:],
                                 start=True, stop=True)
                y = head.tile([128, BL], f32, tag=f"{name}_relu")
                nc.vector.tensor_scalar(
                    out=y[:], in0=y_ps[:],
                    scalar1=par[:, b_col:b_col + 1], scalar2=0.0,
                    op0=mybir.AluOpType.add, op1=mybir.AluOpType.max)
                if lean:
                    return y
                z = head.tile([128, BL], f32, tag=f"{name}_bn")
                nc.vector.tensor_scalar(
                    out=z[:], in0=y[:],
                    scalar1=par[:, bng_col:bng_col + 1],
                    scalar2=par[:, bnb_col:bnb_col + 1],
                    op0=mybir.AluOpType.mult, op1=mybir.AluOpType.add)
                return z

            h1 = layer_norm(x0_ps, 6, "ln1")
            z1 = linear_relu_bn(h1, 12, 0, 2, 3, lean1, "l1")
            z1t_ps = ps_head.tile([BL, 128], f32, tag="psh", name="z1t")
            nc.tensor.transpose(z1t_ps[:], z1[:], ident[:, :])
            h2 = layer_norm(z1t_ps, 7, "ln2")
            z2 = linear_relu_bn(h2, 140, 1, 4, 5, lean2, "l2")

            out_ps = ps_head.tile([BL, 128], f32, tag="psh", name="outT")
            nc.tensor.transpose(out_ps[:], z2[:], ident[:, :])
            out_sb = head.tile([BL, 128], f32)
            nc.vector.tensor_copy(out_sb[:], out_ps[:])
            nc.scalar.dma_start(out_d[:, :], out_sb[:])

    nc.compile()
    return nc


def _prepare_in_maps(inputs):
    import ml_dtypes

    x = np.asarray(inputs["inputs"])
    emb = np.asarray(inputs["emb"], dtype=np.float32)
    w1 = np.asarray(inputs["w1"], dtype=np.float32)
    b1 = np.asarray(inputs["b1"], dtype=np.float32)
    w2 = np.asarray(inputs["w2"], dtype=np.float32)
    b2 = np.asarray(inputs["b2"], dtype=np.float32)
    ln1_g = np.asarray(inputs["ln1_g"], np.float32)
    ln1_b = np.asarray(inputs["ln1_b"], np.float32)
    ln2_g = np.asarray(inputs["ln2_g"], np.float32)
    ln2_b = np.asarray(inputs["ln2_b"], np.float32)

    par = np.zeros((128, NPAR), dtype=np.float32)
    # y = W @ (g*xn + b) + b1 = (W*g) @ xn + (W@b + b1)
    w1f = w1 * ln1_g[None, :]
    b1f = b1 + w1 @ ln1_b
    w2f = w2 * ln2_g[None, :]
    b2f = b2 + w2 @ ln2_b
    bn1_g = np.asarray(inputs["bn1_g"], np.float32) / np.sqrt(
        np.float32(1.0) + np.float32(EPS))
    bn1_b = np.asarray(inputs["bn1_b"], np.float32)
    bn2_g = np.asarray(inputs["bn2_g"], np.float32) / np.sqrt(
        np.float32(1.0) + np.float32(EPS))
    bn2_b = np.asarray(inputs["bn2_b"], np.float32)
    # g>0 and b==0 lets BN fold into the preceding linear: g*relu(Wx+b) =
    # relu((g*W)x + g*b)
    lean1 = bool((bn1_g > 0).all() and (bn1_b == 0).all())
    lean2 = bool((bn2_g > 0).all() and (bn2_b == 0).all())
    if lean1:
        w1f = w1f * bn1_g[:, None]
        b1f = b1f * bn1_g
    if lean2:
        w2f = w2f * bn2_g[:, None]
        b2f = b2f * bn2_g
    par[:, 0] = b1f
    par[:, 1] = b2f
    par[:, 2] = bn1_g
    par[:, 3] = bn1_b
    par[:, 4] = bn2_g
    par[:, 5] = bn2_b
    par[:, 6] = EPS * P * P
    par[:, 7] = EPS
    for i in range(NB):
        par[i, 8 + i // P] = 1.0
    par[:, 12:140] = w1f.T
    par[:, 140:268] = w2f.T

    x_flat = np.asarray(x).reshape(BP, E)
    in_maps = []
    for c in range(NCORES):
        lo = c * E_SH
        seg_t = np.zeros((E_PAD, BP), dtype=np.int8)
        seg_t[:E_SH] = (x_flat[:, lo:lo + E_SH].T == 1)
        x_sh = np.ascontiguousarray(
            seg_t.reshape(NSUB, SUB, BP).transpose(1, 0, 2)
        ).reshape(SUB, NSUB * BP).astype(ml_dtypes.float8_e4m3)
        seg_e = np.zeros((E_PAD, 129), dtype=np.float32)
        seg_e[:E_SH, 0:128] = emb[lo:lo + E_SH, :]
        if c == 0:
            seg_e[0, 0:128] = 0.0   # padding_idx=0
        seg_e[:, 128] = 1.0         # count column
        emb_sh = np.ascontiguousarray(
            seg_e.reshape(NSUB, SUB, 129).transpose(1, 0, 2)
        ).reshape(SUB, NSUB * 129).astype(ml_dtypes.bfloat16)
        in_maps.append({"x": x_sh, "emb": emb_sh, "par": par})
    return in_maps, lean1, lean2


def _run(inputs, trace=False):
    from concourse.bass_utils import run_bass_kernel_spmd

    in_maps, lean1, lean2 = _prepare_in_maps(inputs)
    key = ("nc", lean1, lean2)
    if key not in _cached:
        _cached[key] = _build(lean1, lean2)
    nc = _cached[key]
    res = run_bass_kernel_spmd(
        nc, in_maps, core_ids=list(range(NCORES)), trace=trace)
    out = np.concatenate(
        [np.asarray(res.results[c]["out"]) for c in range(NCORES)], axis=0)
    return out, res.exec_time_ns


def kernel(**inputs) -> np.ndarray:
    out, _ = _run(inputs, trace=False)
    return out


# revision 17
# speedup vs baseline: 1.4626x; 1.2046x over previous
"""Trainium2 Bass kernel for nn_EntityEncoder — batch-parallel, no collectives.

Each core owns 4 batches (64 paths) and reads the FULL vocab for them:
  - x slice [64 paths, 50000] -> transposed/padded [128, 391*64] fp8 (3.2 MB)
  - full [emb | ones] [128, 391*129] bf16 (12.9 MB, SBUF-resident)
One PSUM accumulation [64, 129] over 391 K=128 subtiles gives sums AND
counts (ones column) with no cross-core exchange at all — no ncfw barrier
(~41 us), no ReduceScatter (~11 us). Head is fully local per core.
"""

import numpy as np

B, P, E, H = 32, 16, 50000, 128
NCORES = 8
BP = B * P
EPS = 1e-5
NB = BP // NCORES           # 64 local paths
BL = B // NCORES            # 4 local batches
SUB = 128
E_PAD = 50048
NSUB = E_PAD // SUB         # 391
SPC = 25                    # subtiles per DMA chunk
CHUNKS = [SPC] * 15 + [NSUB - 15 * SPC]   # 15x25 + 16

NPAR = 268  # same packed-params layout as the vocab-sharded kernel

_cached = {}


def _build(lean1=False, lean2=False):
    import concourse.bacc as bacc
    import concourse.mybir as mybir
    import concourse.tile as tile
    from concourse import masks

    f32 = mybir.dt.float32
    bf16 = mybir.dt.bfloat16
    fp8 = mybir.dt.float8e4

    nc = bacc.Bacc("TRN2", target_bir_lowering=False, debug=False,
                   num_devices=NCORES)

    x_d = nc.dram_tensor("x", [SUB, NSUB * NB], fp8, kind="ExternalInput")
    emb_d = nc.dram_tensor("emb", [SUB, NSUB * 129], bf16,
                           kind="ExternalInput")
    par_d = nc.dram_tensor("par", [128, NPAR], f32, kind="ExternalInput")
    out_d = nc.dram_tensor("out", [BL, H], f32, kind="ExternalOutput")

    with tile.TileContext(nc) as tc:
        with tc.tile_pool(name="const", bufs=1) as constp, \
             tc.tile_pool(name="xin", bufs=3) as xin, \
             tc.tile_pool(name="ein", bufs=3) as ein, \
             tc.tile_pool(name="head", bufs=1) as head, \
             tc.tile_pool(name="ps_acc", bufs=1, space="PSUM") as ps_acc, \
             tc.tile_pool(name="ps_head", bufs=4, space="PSUM") as ps_head:

            ident = constp.tile([128, 128], f32)
            masks.make_identity(nc, ident[:])
            par = constp.tile([128, NPAR], f32)
            nc.scalar.dma_start(par[:], par_d[:, :])

            warm = constp.tile([1, 1], f32)
            nc.scalar.activation(warm[:], par[0:1, 7:8],
                                 mybir.ActivationFunctionType.Sqrt,
                                 bias=par[0:1, 7:8], scale=1.0)

            ps0 = ps_acc.tile([NB, 512], f32, name="acc")
            goff = 0
            for t, S in enumerate(CHUNKS):
                et = ein.tile([SUB, S * 129], bf16, tag="et", name=f"et{t}")
                nc.scalar.dma_start(
                    et[:], emb_d[:, goff * 129:(goff + S) * 129])
                xt = xin.tile([SUB, S * NB], fp8, tag="xt", name=f"xt{t}")
                nc.sync.dma_start(
                    xt[:], x_d[:, goff * NB:(goff + S) * NB])
                for j in range(S):
                    g = goff + j
                    nc.tensor.matmul(
                        ps0[:, 0:129],
                        xt[:, j * NB:(j + 1) * NB],
                        et[:, j * 129:(j + 1) * 129],
                        start=(g == 0), stop=(g == NSUB - 1))
                goff += S

            # ---- head on local [64, 129] totals ----
            Ssb = head.tile([NB, 129], f32)
            nc.vector.tensor_copy(Ssb[:], ps0[:, 0:129])

            rec = head.tile([NB, 1], f32)
            nc.vector.reciprocal(rec[:], Ssb[:, 128:129])
            R = head.tile([NB, BL], f32)
            nc.vector.tensor_scalar(
                out=R[:], in0=par[0:NB, 8:12], scalar1=rec[:, 0:1],
                scalar2=None, op0=mybir.AluOpType.mult)

            x0_ps = ps_head.tile([BL, 128], f32, tag="psh", name="x0")
            nc.tensor.matmul(x0_ps[:], R[:], Ssb[:, 0:128],
                             start=True, stop=True)

            def layer_norm(x_ps, eps_col, name):
                st6 = head.tile([BL, 6], f32, tag=f"{name}_st6")
                nc.vector.bn_stats(st6[:], x_ps[:])
                mv = head.tile([BL, 2], f32, tag=f"{name}_mv")
                nc.vector.bn_aggr(mv[:], st6[:])
                sd = head.tile([BL, 1], f32, tag=f"{name}_sd")
                nc.scalar.activation(sd[:], mv[:, 1:2],
                                     mybir.ActivationFunctionType.Sqrt,
                                     bias=par[0:BL, eps_col:eps_col + 1],
                                     scale=1.0)
                rstd = head.tile([BL, 1], f32, tag=f"{name}_rstd")
                nc.vector.reciprocal(rstd[:], sd[:])
                xn = head.tile([BL, 128], f32, tag=f"{name}_xn")
                nc.vector.tensor_scalar(
                    out=xn[:], in0=x_ps[:],
                    scalar1=mv[:, 0:1], scalar2=rstd[:, 0:1],
                    op0=mybir.AluOpType.subtract, op1=mybir.AluOpType.mult)
                return xn

            def linear_relu_bn(xn, w_lo, b_col, bng_col, bnb_col, lean,
                               name):
                xt_ps = ps_head.tile([128, BL], f32, tag="psh",
                                     name=f"{name}_xt")
                nc.tensor.transpose(xt_ps[:], xn[:], ident[0:BL, 0:BL])
                xt_sb = head.tile([128, BL], f32, tag=f"{name}_xts")
                nc.vector.tensor_copy(xt_sb[:], xt_ps[:])
                y_ps = ps_head.tile([128, BL], f32, tag="psh",
                                    name=f"{name}_y")
                nc.tensor.matmul(y_ps[:], par[:, w_lo:w_lo + 128], xt_sb[:],
                                 start=True, stop=True)
                y = head.tile([128, BL], f32, tag=f"{name}_relu")
                nc.vector.tensor_scalar(
                    out=y[:], in0=y_ps[:],
                    scalar1=par[:, b_col:b_col + 1], scalar2=0.0,
                    op0=mybir.AluOpType.add, op1=mybir.AluOpType.max)
                if lean:
                    return y
                z = head.tile([128, BL], f32, tag=f"{name}_bn")
                nc.vector.tensor_scalar(
                    out=z[:], in0=y[:],
                    scalar1=par[:, bng_col:bng_col + 1],
                    scalar2=par[:, bnb_col:bnb_col + 1],
                    op0=mybir.AluOpType.mult, op1=mybir.AluOpType.add)
                return z

            h1 = layer_norm(x0_ps, 6, "ln1")
            z1 = linear_relu_bn(h1, 12, 0, 2, 3, lean1, "l1")
            z1t_ps = ps_head.tile([BL, 128], f32, tag="psh", name="z1t")
            nc.tensor.transpose(z1t_ps[:], z1[:], ident[:, :])
            h2 = layer_norm(z1t_ps, 7, "ln2")
            z2 = linear_relu_bn(h2, 140, 1, 4, 5, lean2, "l2")

            out_ps = ps_head.tile([BL, 128], f32, tag="psh", name="outT")
            nc.tensor.transpose(out_ps[:], z2[:], ident[:, :])
            out_sb = head.tile([BL, 128], f32)
            nc.vector.tensor_copy(out_sb[:], out_ps[:])
            nc.scalar.dma_start(out_d[:, :], out_sb[:])

    nc.compile()
    return nc


def _prepare_in_maps(inputs):
    import ml_dtypes

    x = np.asarray(inputs["inputs"])
    emb = np.asarray(inputs["emb"], dtype=np.float32)
    w1 = np.asarray(inputs["w1"], dtype=np.float32)
    b1 = np.asarray(inputs["b1"], dtype=np.float32)
    w2 = np.asarray(inputs["w2"], dtype=np.float32)
    b2 = np.asarray(inputs["b2"], dtype=np.float32)
    ln1_g = np.asarray(inputs["ln1_g"], np.float32)
    ln1_b = np.asarray(inputs["ln1_b"], np.float32)
    ln2_g = np.asarray(inputs["ln2_g"], np.float32)
    ln2_b = np.asarray(inputs["ln2_b"], np.float32)

    par = np.zeros((128, NPAR), dtype=np.float32)
    w1f = w1 * ln1_g[None, :]
    b1f = b1 + w1 @ ln1_b
    w2f = w2 * ln2_g[None, :]
    b2f = b2 + w2 @ ln2_b
    bn1_g = np.asarray(inputs["bn1_g"], np.float32) / np.sqrt(
        np.float32(1.0) + np.float32(EPS))
    bn1_b = np.asarray(inputs["bn1_b"], np.float32)
    bn2_g = np.asarray(inputs["bn2_g"], np.float32) / np.sqrt(
        np.float32(1.0) + np.float32(EPS))
    bn2_b = np.asarray(inputs["bn2_b"], np.float32)
    lean1 = bool((bn1_g > 0).all() and (bn1_b == 0).all())
    lean2 = bool((bn2_g > 0).all() and (bn2_b == 0).all())
    if lean1:
        w1f = w1f * bn1_g[:, None]
        b1f = b1f * bn1_g
    if lean2:
        w2f = w2f * bn2_g[:, None]
        b2f = b2f * bn2_g
    par[:, 0] = b1f
    par[:, 1] = b2f
    par[:, 2] = bn1_g
    par[:, 3] = bn1_b
    par[:, 4] = bn2_g
    par[:, 5] = bn2_b
    par[:, 6] = EPS * P * P
    par[:, 7] = EPS
    for i in range(NB):
        par[i, 8 + i // P] = 1.0
    par[:, 12:140] = w1f.T
    par[:, 140:268] = w2f.T

    # shared [emb | ones] in subtile-major bf16, built once
    seg_e = np.zeros((E_PAD, 129), dtype=np.float32)
    seg_e[:E, 0:128] = emb
    seg_e[0, 0:128] = 0.0       # padding_idx=0
    seg_e[:, 128] = 1.0
    emb_sh = np.ascontiguousarray(
        seg_e.reshape(NSUB, SUB, 129).transpose(1, 0, 2)
    ).reshape(SUB, NSUB * 129).astype(ml_dtypes.bfloat16)

    x_flat = np.asarray(x).reshape(BP, E)
    in_maps = []
    for c in range(NCORES):
        seg_t = np.zeros((E_PAD, NB), dtype=np.int8)
        seg_t[:E] = (x_flat[c * NB:(c + 1) * NB, :].T == 1)
        x_sh = np.ascontiguousarray(
            seg_t.reshape(NSUB, SUB, NB).transpose(1, 0, 2)
        ).reshape(SUB, NSUB * NB).astype(ml_dtypes.float8_e4m3)
        in_maps.append({"x": x_sh, "emb": emb_sh, "par": par})
    return in_maps, lean1, lean2


def _run(inputs, trace=False):
    from concourse.bass_utils import run_bass_kernel_spmd

    in_maps, lean1, lean2 = _prepare_in_maps(inputs)
    key = ("nc", lean1, lean2)
    if key not in _cached:
        _cached[key] = _build(lean1, lean2)
    nc = _cached[key]
    res = run_bass_kernel_spmd(
        nc, in_maps, core_ids=list(range(NCORES)), trace=trace)
    out = np.concatenate(
        [np.asarray(res.results[c]["out"]) for c in range(NCORES)], axis=0)
    return out, res.exec_time_ns


def kernel(**inputs) -> np.ndarray:
    out, _ = _run(inputs, trace=False)
    return out
